# revision 32
# baseline (speedup 1.0000x reference)
"""Trainium2 Bass kernel for nn_AutoPruneNet (MLP policy/baseline heads + sampling).

Math (per row r of TB = T*B rows):
    h1 = relu(x @ W1.T + b1)            x: [512], h1: [400]
    h2 = relu(h1 @ W2.T + b2)           h2: [300]
    core = [h2, clip(reward,-1,1), last_action]   [302]
    pl = sigmoid(core @ Wp.T + bp)      [2]  (mu, sigma)
    baseline = core @ Wb.T + bb         [1]
    action = pl0 + pl1 * eps
    out[r] = [pl0, pl1, baseline, action]

Distribution: pure data parallel, TB rows split contiguously across 8 cores
(16384 rows each); weights replicated.

Precision: fp8(e4m3) activations + weights with DoubleRow matmuls (2 fp8
weights per PE cell -> K=256 per stream), roughly halving PE streams vs bf16.
Weights are scaled x8 on host so they sit in e4m3's normal range; the scale
compounds through the layers (psum1 = 8*y1, psum2 = 64*y2, psum_h = 64*z) and
is divided back out for free via the ACT engine's input `scale` operand.
Activations are stored as 8*h in fp8.

Layout: fc1/fc2 run feature-major [feature, row] (contraction on partitions,
zero-padded to 512 where needed — K padding costs no PE time, stream cost
depends only on N=512). The HEAD runs row-major: lhsT = core slice
[K, 128 rows] (stationary), rhs = head weights [K, 4] (moving), so the head
psum is [128 rows, (mu,sigma,base,pad)] and the whole sampling epilogue is a
handful of partition-parallel [128, 16, *] ops per 4-tile group instead of
one-partition [1,512] ops. Head biases ride as a constant-8.0 row appended to
the rwla DMA (core row 98) with 8*b head-weight entries.

Schedule: fc2 of tile t-1 is emitted after fc1 of tile t (so fc2 never waits
on the same tile's DVE relus); head phase of tile u runs at iteration u+2.
All constant inputs (weights/biases/eps) are fused into one [128, 3440]-byte
DMA (DMA preamble is packet-rate bound, ~1 packet per partition per
instruction).
"""
import sys
import types

import numpy as np
import ml_dtypes

import concourse.bacc as bacc
import concourse.bass as bass
import concourse.mybir as mybir
import concourse.tile as tile
from concourse.bass import ds, ts
from concourse.bass_utils import run_bass_kernel_spmd


def _install_ntff_hook_shim():
    """Provide the optional antenv.axon_hooks module if the image lacks it,
    so a BASS_TRACE env var in the caller can't crash run_bass_kernel_spmd.
    Registers the real NTFF profile hook when the axon .so supports it."""
    try:
        import antenv.axon_hooks  # noqa: F401
        return
    except Exception:
        pass
    try:
        import antenv
    except Exception:
        return
    mod = types.ModuleType("antenv.axon_hooks")
    state = {"hook": None}
    mod.set_axon_ntff_profile_hook = lambda h: state.__setitem__("hook", h)
    mod.get_axon_ntff_profile_hook = lambda: state["hook"]
    sys.modules["antenv.axon_hooks"] = mod
    antenv.axon_hooks = mod
    try:
        from trn_agent_boot.trn_boot import _ntff_profile_via_ctypes
        mod.set_axon_ntff_profile_hook(
            _ntff_profile_via_ctypes('/opt/axon/libaxon_pjrt.so'))
    except Exception:
        pass


_install_ntff_hook_shim()

E4 = ml_dtypes.float8_e4m3fn

N_CORES = 8
T, B, OBS = 64, 2048, 512
H1, H2 = 400, 300
TB = T * B
R = TB // N_CORES       # rows per core
NT = 512                # rows per row-tile (matmul moving dim)
OG = 4                  # row-tiles per output group (tail groups taper 2,1,1)
RC = NT // 128          # 128-row head chunks per tile (4)
GC = OG * RC            # head chunks per full group (16)
N_WARM = 6              # HAM warm-up matmuls issued during the DMA preamble


def group_layout(n_tiles):
    """Group sizes: full OG-sized groups, tapered tail (2,1,1) so the final
    epilogue chain after the last matmul is short. Returns per-tile
    (group, pos_in_group, group_size, base_chunk)."""
    assert n_tiles % OG == 0 and n_tiles >= 2 * OG
    sizes = [OG] * (n_tiles // OG - 1) + [2, 2]
    per_tile = []
    base = 0
    for g, og in enumerate(sizes):
        for b in range(og):
            per_tile.append((g, b, og, base))
        base += og * RC
    return per_tile

F32 = mybir.dt.float32
F8 = mybir.dt.float8e4
AF = mybir.ActivationFunctionType
ALU = mybir.AluOpType
DR = mybir.MatmulPerfMode.DoubleRow

# fused constant-blob byte offsets (per partition); must match host_prep
OFF_W1 = 0          # [2, 2, 416] fp8
OFF_W2 = 1664       # [2, 2, 304] fp8
OFF_WH1 = 2880      # [2, 4] fp8
OFF_WH2 = 2888      # [4] fp8 (partitions 0..98 used)
OFF_B1 = 2896       # [4] f32
OFF_B2 = 2912       # [3] f32
OFF_EPS = 2928      # [rows//128] f32
WBYTES = 3440

# fc1 output (h1) chunking: {128,128,128,32}; last chunk is 16 real rows of
# h1 plus 16 zero-pad rows (weights zero) so the once-memset zero region of
# h1b's j=1 half starts at partition 32.
M1 = [(0, 128), (128, 128), (256, 128), (384, 32)]
# fc2 output (h2) chunking: {128, 128, 44}
M2 = [(0, 128), (128, 128), (256, 44)]


def build_bass(rows: int):
    """Build the per-core Bass program for `rows` rows (rows % (NT*OG) == 0)."""
    assert rows % (NT * OG) == 0
    assert rows // 128 == (WBYTES - OFF_EPS) // 4
    n_tiles = rows // NT

    nc = bacc.Bacc("TRN2", target_bir_lowering=False, debug=False)

    xt_d = nc.dram_tensor("xt", [128, n_tiles, 2, 2, NT], F8,
                          kind="ExternalInput")
    rwla_d = nc.dram_tensor("rwla", [3, rows], F8, kind="ExternalInput")
    w8_d = nc.dram_tensor("w8", [128, WBYTES], F8, kind="ExternalInput")
    out_d = nc.dram_tensor("out", [128, rows // 128, 4], F32,
                           kind="ExternalOutput")

    with tile.TileContext(nc) as tc:
        with (
            tc.tile_pool(name="w", bufs=1) as wpool,
            tc.tile_pool(name="x", bufs=8) as xpool,
            tc.tile_pool(name="h1a", bufs=6) as h1apool,
            tc.tile_pool(name="c1", bufs=8) as c1pool,
            tc.tile_pool(name="ot", bufs=4) as opool,
            tc.tile_pool(name="s", bufs=4) as spool,
            tc.tile_pool(name="ps1", bufs=1, space="PSUM") as ppool1,
            tc.tile_pool(name="ps2", bufs=1, space="PSUM") as ppool2,
            tc.tile_pool(name="ps3", bufs=1, space="PSUM") as ppool3,
        ):
            w8 = wpool.tile([128, WBYTES], F8, tag="w8")
            # w1 k=0 slice first: the first fc1 matmul needs only it + the
            # k=0 half of xt(0)
            nc.scalar.dma_start(w8[:, 0:832], w8_d[:, 0:832])
            nc.scalar.dma_start(w8[:, 832:OFF_W2], w8_d[:, 832:OFF_W2])
            nc.scalar.dma_start(w8[:, OFF_W2:], w8_d[:, OFF_W2:])

            # Fixed-bank PSUM tiles: one bank per fc1 chunk (4) / fc2 chunk
            # (3) / head group (1). Pool-rotated psum tiles hand the
            # just-freed bank to the next chunk, making every matmul WAR-wait
            # on the relu that read that bank ~750ns earlier; a fixed
            # bank-per-chunk gives a full tile (~3.4us) of slack instead.
            ps1_t = [ppool1.tile([128, NT], F32, tag=f"ps1_{i}",
                                 name=f"ps1_{i}") for i in range(4)]
            ps2_t = [ppool2.tile([128, NT], F32, tag=f"ps2_{i}",
                                 name=f"ps2_{i}") for i in range(3)]
            # two half-bank head-psum slots in one bank, alternating per group
            psh2 = ppool3.tile([128, 2, GC, 4], F32, tag="ps3", name="psh2")

            # HAM warm-up: the PE clock idles at 1.2 GHz and only reaches
            # 2.4 GHz after ~3.4us of sustained activity. Run dummy matmuls
            # on a zeroed scratch tile while the weight/xt DMAs are in
            # flight so the real matmuls start at full clock.
            scr = wpool.tile([128, 640], F8, tag="scr")
            nc.gpsimd.memset(scr[:, :], 0.0)
            for i in range(N_WARM):
                nc.tensor.matmul(ps2_t[i % 3][:, :], scr[:, 0:128],
                                 scr[:, 128:640], start=True, stop=True)
            w1_sb = w8[:, OFF_W1:OFF_W1 + 1664].rearrange(
                "p (k j m) -> p k j m", k=2, j=2, m=416)
            w2_sb = w8[:, OFF_W2:OFF_W2 + 1216].rearrange(
                "p (k j m) -> p k j m", k=2, j=2, m=304)
            wh1_sb = w8[:, OFF_WH1:OFF_WH1 + 8].rearrange(
                "p (j m) -> p j m", j=2, m=4)
            wh2_sb = w8[0:99, OFF_WH2:OFF_WH2 + 4]
            b1_sb = w8[:, OFF_B1:OFF_B1 + 16].bitcast(F32)      # [128, 4]
            b2_sb = w8[:, OFF_B2:OFF_B2 + 12].bitcast(F32)      # [128, 3]
            eps_sb = w8[:, OFF_EPS:WBYTES].bitcast(F32)         # [128, r/128]

            # Persistent rotating buffers whose zero regions are memset ONCE:
            #  h1b: j=0 -> h1 chunk2 (rewritten each tile); j=1 partitions
            #       0..31 -> h1 chunk3 (rewritten; rows 16..31 zero via zero
            #       weights); j=1 partitions 32..127 -> zero forever.
            #  c2:  99 partitions: [0:44] h2 chunk (rewritten), [44:96] zero
            #       forever, [96:99] (cr, la, const-8) DMA'd each tile.
            NB1 = 6
            NB2 = 10
            h1b_bufs, c2_bufs = [], []
            for i in range(NB1):
                hb = wpool.tile([128, 2, NT], F8, tag=f"h1b{i}")
                nc.gpsimd.memset(hb[32:64, 1, :], 0.0)
                nc.gpsimd.memset(hb[64:128, 1, :], 0.0)
                h1b_bufs.append(hb)
            for i in range(NB2):
                cb = wpool.tile([99, NT], F8, tag=f"c2{i}")
                nc.gpsimd.memset(cb[32:64, :], 0.0)
                nc.gpsimd.memset(cb[64:96, :], 0.0)
                c2_bufs.append(cb)

            utiles = {}     # tile u -> (c1, c2) for the head
            gps = {}        # group g -> (psh, obt)
            gmap = group_layout(n_tiles)
            pending_out = []

            def flush_out(now=None):
                # defer each out descriptor ~6 iterations: the sync engine
                # runs that far ahead of compute, so by the time it reaches
                # the descriptor the epilogue it waits on has completed and
                # it can't head-of-line block the xt descriptors behind it
                while pending_out and (now is None
                                       or pending_out[0][0] <= now - 6):
                    _, osl, obt = pending_out.pop(0)
                    nc.sync.dma_start(out_d[:, osl, :], obt[:])

            def emit_head_phase(u):
                """Emit head chunks for the rows of tile u; after the last
                phase of a group, the sampling epilogue + out DMA."""
                g, b, og, base = gmap[u]
                gc = og * RC
                c1, c2 = utiles.pop(u)
                psh = psh2[:, g % 2]
                if b == 0:
                    gps[g] = opool.tile([128, gc, 4], F32, tag="obt",
                                        name="obt")
                obt = gps[g]
                for q in range(RC):
                    c = RC * b + q
                    rsl = ds(q * 128, 128)
                    nc.tensor.matmul(psh[:, c, :], c2[:, rsl], wh2_sb,
                                     start=True, stop=False)
                    nc.tensor.matmul(psh[:, c, :], c1[:, 0, rsl],
                                     wh1_sb[:, 0, :], start=False, stop=False)
                    nc.tensor.matmul(psh[:, c, :], c1[:, 1, rsl],
                                     wh1_sb[:, 1, :], start=False, stop=True)
                if b == og - 1:
                    # psum = 64*(z + b);  pl = sigmoid(z + b) etc.
                    nc.scalar.activation(obt[:, :, 0:2], psh[:, 0:gc, 0:2],
                                         AF.Sigmoid, scale=1.0 / 64.0)
                    nc.vector.tensor_scalar_mul(obt[:, :, 2], psh[:, 0:gc, 2],
                                                1.0 / 64.0)
                    se = spool.tile([128, gc], F32, tag="se")
                    nc.vector.tensor_mul(se[:], obt[:, :, 1],
                                         eps_sb[:, ds(base, gc)])
                    nc.vector.tensor_add(obt[:, :, 3], obt[:, :, 0], se[:])
                    # deferred: emitted on the sync queue next iteration so
                    # the sigmoid-gated descriptor can't head-of-line block
                    # the xt descriptor right behind it
                    pending_out.append((u + 2, ds(base, gc), obt))
                    del gps[g]

            def emit_fc2(t, h1a, h1b, c1, c2):
                # fc2: h2T chunks {128, 128, 44}; psum = 64*y2; the m=2
                # chunk goes first so c2's assembly (relu + rwla DMA)
                # finishes earliest
                for m in (2, 0, 1):
                    m0, mw = M2[m]
                    ps2 = ps2_t[m]
                    for k in range(2):
                        rhs = h1a if k == 0 else h1b
                        nc.tensor.matmul(
                            ps2[0:mw, :],
                            w2_sb[:, k, :, ds(m0, mw)],
                            rhs[:, :, :],
                            start=(k == 0),
                            stop=(k == 1),
                            perf_mode=DR,
                        )
                    # relu(64y2/8 + 8b2) on ACT -> 8*h2 in fp8
                    if m < 2:
                        nc.scalar.activation(c1[:, m, :], ps2[0:mw, :],
                                             AF.Relu,
                                             bias=b2_sb[0:mw, m:m + 1],
                                             scale=0.125)
                    else:
                        nc.scalar.activation(c2[0:44, :], ps2[0:mw, :],
                                             AF.Relu,
                                             bias=b2_sb[0:mw, m:m + 1],
                                             scale=0.125)
                utiles[t] = (c1, c2)

            fc1_out = {}    # tile t -> (h1a, h1b) for the lagged fc2
            fc2_in = {}     # tile t -> (c1, c2)

            for t in range(n_tiles + 2):
                if t > 0:
                    flush_out(t)
                if t < n_tiles:
                    xt_t = xpool.tile([128, 2, 2, NT], F8, tag="xt")
                    if t == 0:
                        # split halves so the k=0 matmuls can start as soon
                        # as the first half lands
                        nc.sync.dma_start(xt_t[:, 0, :, :],
                                          xt_d[:, t, 0, :, :])
                        nc.sync.dma_start(xt_t[:, 1, :, :],
                                          xt_d[:, t, 1, :, :])
                    elif t == 1:
                        # parallel queue: at ramp the single sync DMA queue
                        # (~107 GB/s) can't deliver xt(0)+xt(1) in time
                        nc.gpsimd.dma_start(xt_t[:], xt_d[:, t, :, :, :])
                    else:
                        nc.sync.dma_start(xt_t[:], xt_d[:, t, :, :, :])
                    h1b = h1b_bufs[t % NB1]
                    c2 = c2_bufs[t % NB2]
                    nc.sync.dma_start(c2[96:99, :], rwla_d[:, ts(t, NT)])

                    # fc1: h1T chunks {128,128,128,32}; psum = 8*y1. Tile 0
                    # runs all k=0 matmuls first (they only need the first
                    # halves of the w1/xt DMAs).
                    h1a = h1apool.tile([128, 2, NT], F8, tag="h1a")
                    korder = ([(k, c) for k in range(2) for c in range(4)]
                              if t == 0 else
                              [(k, c) for c in range(4) for k in range(2)])
                    for ki, (k, c) in enumerate(korder):
                        if t == 0 and ki in (4, 6):
                            # ramp filler: keep the PE array active while the
                            # xt(0) k=1 half / xt(1) DMAs land, so the HAM
                            # clock gate doesn't re-throttle mid-ramp
                            for _ in range(2):
                                nc.tensor.matmul(ps2_t[2][:, :], scr[:, 0:128],
                                                 scr[:, 128:640],
                                                 start=True, stop=True)
                        m0, mw = M1[c]
                        ps = ps1_t[c]
                        nc.tensor.matmul(
                            ps[0:mw, :],
                            w1_sb[:, k, :, ds(m0, mw)],
                            xt_t[:, k, :, :],
                            start=(k == 0),
                            stop=(k == 1),
                            perf_mode=DR,
                        )
                        if k != 1:
                            continue
                        # relu((8y1) + 8b1) -> 8*h1 in fp8; the small
                        # chunk goes to ACT to offload the DVE
                        if c < 2:
                            dest = h1a[:, c, :]
                        elif c == 2:
                            dest = h1b[:, 0, :]
                        else:
                            dest = h1b[0:32, 1, :]
                        if c < 3:
                            nc.vector.tensor_scalar(
                                dest, ps[0:mw, :], b1_sb[0:mw, c:c + 1], 0.0,
                                ALU.add, ALU.max
                            )
                        else:
                            nc.scalar.activation(
                                dest, ps[0:mw, :], AF.Relu,
                                bias=b1_sb[0:mw, c:c + 1])
                    fc1_out[t] = (h1a, h1b)
                    fc2_in[t] = (c1pool.tile([128, 2, NT], F8, tag="c1",
                                             name="c1"), c2)

                # head of tile t-2, interleaved between fc1(t) and fc2(t-1)
                if t >= 2:
                    emit_head_phase(t - 2)

                if 1 <= t <= n_tiles:
                    h1a_p, h1b_p = fc1_out.pop(t - 1)
                    c1_p, c2_p = fc2_in.pop(t - 1)
                    emit_fc2(t - 1, h1a_p, h1b_p, c1_p, c2_p)
            flush_out()

    nc.compile()
    return nc


def host_prep(frame, reward, last_action, eps, W1, b1, W2, b2, Wp, bp, Wb, bb,
              rows=R, n_cores=N_CORES):
    """Shard + lay out inputs for the device program. Returns in_maps."""
    frame = np.asarray(frame, np.float32).reshape(TB, OBS)
    reward = np.asarray(reward, np.float32).reshape(TB)
    la = np.asarray(last_action).reshape(TB).astype(np.float32)
    eps = np.asarray(eps, np.float32).reshape(TB)
    n_tiles = rows // NT

    W1 = np.asarray(W1, np.float32)
    W2 = np.asarray(W2, np.float32)
    b1 = np.asarray(b1, np.float32)
    b2 = np.asarray(b2, np.float32)
    Wp = np.asarray(Wp, np.float32)
    bp = np.asarray(bp, np.float32)
    Wb = np.asarray(Wb, np.float32)
    bb = np.asarray(bb, np.float32)

    # frame features f are split as f = 256k + 128j + ki
    frame_q = frame.astype(E4)          # one pass over the big tensor
    W1p = np.zeros((416, 512), np.float32)
    W1p[0:400] = 8.0 * W1
    w1_h = np.ascontiguousarray(
        W1p.T.reshape(2, 2, 128, 416).transpose(2, 0, 1, 3)).astype(E4)
    W2p = np.zeros((304, 512), np.float32)
    W2p[0:300, 0:400] = 8.0 * W2
    w2_h = np.ascontiguousarray(
        W2p.T.reshape(2, 2, 128, 304).transpose(2, 0, 1, 3)).astype(E4)

    # head weights, row-major heads: columns (mu, sigma, baseline, pad);
    # core rows: 0..255 (c1: f = 128j + ki), then c2 rows {0..43: h2
    # 256..299, 44..95: zero, 96: cr, 97: la, 98: const-8 bias row}
    Wh = np.concatenate([Wp, Wb], axis=0)           # [3, 302]
    bh = np.array([bp[0], bp[1], bb[0]], np.float32)
    wh1_h = np.zeros((128, 2, 4), np.float32)
    wh1_h[:, :, 0:3] = (8.0 * Wh[:, 0:256]).T.reshape(2, 128, 3).transpose(
        1, 0, 2)
    wh1_h = wh1_h.astype(E4)
    wh2_h = np.zeros((128, 4), np.float32)
    wh2_h[0:44, 0:3] = 8.0 * Wh[:, 256:300].T
    wh2_h[96:98, 0:3] = 8.0 * Wh[:, 300:302].T
    wh2_h[98, 0:3] = 8.0 * bh
    wh2_h = wh2_h.astype(E4)

    b1s = np.zeros(512, np.float32)
    b1s[0:400] = 8.0 * b1
    b1_h = np.ascontiguousarray(b1s.reshape(4, 128).T)
    b2s = np.zeros(384, np.float32)
    b2s[0:300] = 8.0 * b2
    b2_h = np.ascontiguousarray(b2s.reshape(3, 128).T)

    # fused constant blob (bytes), shared across cores except eps
    wbuf = np.zeros((128, WBYTES), np.uint8)
    wbuf[:, OFF_W1:OFF_W1 + 1664] = w1_h.reshape(128, 1664).view(np.uint8)
    wbuf[:, OFF_W2:OFF_W2 + 1216] = w2_h.reshape(128, 1216).view(np.uint8)
    wbuf[:, OFF_WH1:OFF_WH1 + 8] = wh1_h.reshape(128, 8).view(np.uint8)
    wbuf[:, OFF_WH2:OFF_WH2 + 4] = wh2_h.view(np.uint8)
    wbuf[:, OFF_B1:OFF_B1 + 16] = b1_h.view(np.uint8)
    wbuf[:, OFF_B2:OFF_B2 + 12] = b2_h.view(np.uint8)

    cr8 = (8.0 * np.clip(reward, -1.0, 1.0)).astype(E4)
    la8 = (8.0 * la).astype(E4)
    ones8 = np.full(TB, 8.0, np.float32).astype(E4)

    in_maps = []
    for c in range(n_cores):
        sl = slice(c * rows, (c + 1) * rows)
        xt = np.ascontiguousarray(
            frame_q[sl].T.reshape(2, 2, 128, n_tiles, NT)
            .transpose(2, 3, 0, 1, 4))
        rwla = np.stack([cr8[sl], la8[sl], ones8[sl]], axis=0)
        # eps row r lives at [r % 128, r // 128]
        eps_c = np.ascontiguousarray(eps[sl].reshape(rows // 128, 128).T)
        wb = wbuf.copy()
        wb[:, OFF_EPS:WBYTES] = eps_c.view(np.uint8)
        in_maps.append({
            "xt": xt,
            "rwla": rwla,
            "w8": wb.view(E4),
        })
    return in_maps


def assemble_out(per_core_outs):
    """[128, R//128, 4] per core (row r at [r%128, r//128]) -> [T, B, 4]."""
    outs = []
    for o in per_core_outs:
        o = np.asarray(o)
        outs.append(o.transpose(1, 0, 2).reshape(-1, B, 4))
    return np.ascontiguousarray(
        np.concatenate(outs, axis=0).astype(np.float32))


_NC_CACHE = {}


def kernel(**inputs) -> np.ndarray:
    in_maps = host_prep(**inputs)
    if R not in _NC_CACHE:
        _NC_CACHE[R] = build_bass(R)
    nc = _NC_CACHE[R]
    res = run_bass_kernel_spmd(nc, in_maps, core_ids=list(range(N_CORES)))
    return assemble_out([res.results[c]["out"] for c in range(N_CORES)])



# revision 37
# speedup vs baseline: 1.1160x; 1.1160x over previous
"""Trainium2 Bass kernel for nn_AutoPruneNet (MLP policy/baseline heads + sampling).

Math (per row r of TB = T*B rows):
    h1 = relu(x @ W1.T + b1)            x: [512], h1: [400]
    h2 = relu(h1 @ W2.T + b2)           h2: [300]
    core = [h2, clip(reward,-1,1), last_action]   [302]
    pl = sigmoid(core @ Wp.T + bp)      [2]  (mu, sigma)
    baseline = core @ Wb.T + bb         [1]
    action = pl0 + pl1 * eps
    out[r] = [pl0, pl1, baseline, action]

Distribution: pure data parallel, TB rows split contiguously across 8 cores
(16384 rows each); weights replicated.

Precision: fp8(e4m3) activations + weights with DoubleRow matmuls (2 fp8
weights per PE cell -> K=256 per stream), roughly halving PE streams vs bf16.
Weights are scaled x8 on host so they sit in e4m3's normal range; the scale
compounds through the layers (psum1 = 8*y1, psum2 = 64*y2, psum_h = 64*z) and
is divided back out for free via the ACT engine's input `scale` operand.
Activations are stored as 8*h in fp8.

Layout: fc1/fc2 run feature-major [feature, row] (contraction on partitions,
zero-padded to 512 where needed — K padding costs no PE time, stream cost
depends only on N=512). The HEAD runs row-major: lhsT = core slice
[K, 128 rows] (stationary), rhs = head weights [K, 4] (moving), so the head
psum is [128 rows, (mu,sigma,base,pad)] and the whole sampling epilogue is a
handful of partition-parallel [128, 16, *] ops per 4-tile group instead of
one-partition [1,512] ops. Head biases ride as a constant-8.0 row appended to
the rwla DMA (core row 98) with 8*b head-weight entries.

Schedule: fc2 of tile t-1 is emitted after fc1 of tile t (so fc2 never waits
on the same tile's DVE relus); head phase of tile u runs at iteration u+2.
All constant inputs (weights/biases/eps) are fused into one [128, 3440]-byte
DMA (DMA preamble is packet-rate bound, ~1 packet per partition per
instruction).
"""
import sys
import types

import numpy as np
import ml_dtypes

import concourse.bacc as bacc
import concourse.bass as bass
import concourse.mybir as mybir
import concourse.tile as tile
from concourse.bass import ds, ts
from concourse.bass_utils import run_bass_kernel_spmd


def _install_ntff_hook_shim():
    """Provide the optional antenv.axon_hooks module if the image lacks it,
    so a BASS_TRACE env var in the caller can't crash run_bass_kernel_spmd.
    Registers the real NTFF profile hook when the axon .so supports it."""
    try:
        import antenv.axon_hooks  # noqa: F401
        return
    except Exception:
        pass
    try:
        import antenv
    except Exception:
        return
    mod = types.ModuleType("antenv.axon_hooks")
    state = {"hook": None}
    mod.set_axon_ntff_profile_hook = lambda h: state.__setitem__("hook", h)
    mod.get_axon_ntff_profile_hook = lambda: state["hook"]
    sys.modules["antenv.axon_hooks"] = mod
    antenv.axon_hooks = mod
    try:
        from trn_agent_boot.trn_boot import _ntff_profile_via_ctypes
        mod.set_axon_ntff_profile_hook(
            _ntff_profile_via_ctypes('/opt/axon/libaxon_pjrt.so'))
    except Exception:
        pass


_install_ntff_hook_shim()

E4 = ml_dtypes.float8_e4m3fn

N_CORES = 8
T, B, OBS = 64, 2048, 512
H1, H2 = 400, 300
TB = T * B
R = TB // N_CORES       # rows per core
NT = 512                # rows per row-tile (matmul moving dim)
OG = 4                  # row-tiles per output group (tail groups taper 2,1,1)
RC = NT // 128          # 128-row head chunks per tile (4)
GC = OG * RC            # head chunks per full group (16)
N_WARM = 6              # HAM warm-up matmuls issued during the DMA preamble


def group_layout(n_tiles):
    """Group sizes: full OG-sized groups, tapered tail (2,1,1) so the final
    epilogue chain after the last matmul is short. Returns per-tile
    (group, pos_in_group, group_size, base_chunk)."""
    assert n_tiles % OG == 0 and n_tiles >= 2 * OG
    sizes = [OG] * (n_tiles // OG - 1) + [2, 2]
    per_tile = []
    base = 0
    for g, og in enumerate(sizes):
        for b in range(og):
            per_tile.append((g, b, og, base))
        base += og * RC
    return per_tile

F32 = mybir.dt.float32
F8 = mybir.dt.float8e4
AF = mybir.ActivationFunctionType
ALU = mybir.AluOpType
DR = mybir.MatmulPerfMode.DoubleRow

# fused constant-blob byte offsets (per partition); must match host_prep
OFF_W1 = 0          # [2, 2, 416] fp8
OFF_W2 = 1664       # [2, 2, 304] fp8
OFF_WH1 = 2880      # [2, 4] fp8
OFF_WH2 = 2888      # [4] fp8 (partitions 0..98 used)
OFF_B1 = 2896       # [4] f32
OFF_B2 = 2912       # [3] f32
OFF_EPS = 2928      # [rows//128] f32
WBYTES = 3440

# fc1 output (h1) chunking: {128,128,128,32}; last chunk is 16 real rows of
# h1 plus 16 zero-pad rows (weights zero) so the once-memset zero region of
# h1b's j=1 half starts at partition 32.
M1 = [(0, 128), (128, 128), (256, 128), (384, 32)]
# fc2 output (h2) chunking: {128, 128, 44}
M2 = [(0, 128), (128, 128), (256, 44)]


def build_bass(rows: int):
    """Build the per-core Bass program for `rows` rows (rows % (NT*OG) == 0)."""
    assert rows % (NT * OG) == 0
    assert rows // 128 == (WBYTES - OFF_EPS) // 4
    n_tiles = rows // NT

    nc = bacc.Bacc("TRN2", target_bir_lowering=False, debug=False)

    xt_d = nc.dram_tensor("xt", [128, n_tiles, 2, 2, NT], F8,
                          kind="ExternalInput")
    rwla_d = nc.dram_tensor("rwla", [3, rows], F8, kind="ExternalInput")
    w8_d = nc.dram_tensor("w8", [128, WBYTES], F8, kind="ExternalInput")
    out_d = nc.dram_tensor("out", [128, rows // 128, 4], F32,
                           kind="ExternalOutput")

    with tile.TileContext(nc) as tc:
        with (
            tc.tile_pool(name="w", bufs=1) as wpool,
            tc.tile_pool(name="x", bufs=8) as xpool,
            tc.tile_pool(name="h1a", bufs=6) as h1apool,
            tc.tile_pool(name="c1", bufs=8) as c1pool,
            tc.tile_pool(name="ot", bufs=4) as opool,
            tc.tile_pool(name="s", bufs=4) as spool,
            tc.tile_pool(name="ps1", bufs=1, space="PSUM") as ppool1,
            tc.tile_pool(name="ps2", bufs=1, space="PSUM") as ppool2,
            tc.tile_pool(name="ps3", bufs=1, space="PSUM") as ppool3,
        ):
            w8 = wpool.tile([128, WBYTES], F8, tag="w8")
            # w1 k=0 slice first: the first fc1 matmul needs only it + the
            # k=0 half of xt(0)
            nc.scalar.dma_start(w8[:, 0:832], w8_d[:, 0:832])
            nc.scalar.dma_start(w8[:, 832:OFF_W2], w8_d[:, 832:OFF_W2])
            nc.scalar.dma_start(w8[:, OFF_W2:], w8_d[:, OFF_W2:])

            # Fixed-bank PSUM tiles: one bank per fc1 chunk (4) / fc2 chunk
            # (3) / head group (1). Pool-rotated psum tiles hand the
            # just-freed bank to the next chunk, making every matmul WAR-wait
            # on the relu that read that bank ~750ns earlier; a fixed
            # bank-per-chunk gives a full tile (~3.4us) of slack instead.
            ps1_t = [ppool1.tile([128, NT], F32, tag=f"ps1_{i}",
                                 name=f"ps1_{i}") for i in range(4)]
            ps2_t = [ppool2.tile([128, NT], F32, tag=f"ps2_{i}",
                                 name=f"ps2_{i}") for i in range(3)]
            # two half-bank head-psum slots in one bank, alternating per group
            psh2 = ppool3.tile([128, 2, GC, 4], F32, tag="ps3", name="psh2")

            # Prefetch the first two x tiles before anything else queues:
            # xt(0) halves on the sync queue (the k=0 matmuls start after the
            # first half), xt(1) in parallel on the gpsimd queue (one queue
            # moves ~107 GB/s, not enough for both during ramp). These must
            # precede the zero-region memsets below: each memset occupies the
            # gpsimd queue for ~0.5us.
            xt_pre = [xpool.tile([128, 2, 2, NT], F8, tag="xt", name="xt_pre")
                      for _ in range(2)]
            nc.sync.dma_start(xt_pre[0][:, 0, :, :], xt_d[:, 0, 0, :, :])
            nc.sync.dma_start(xt_pre[0][:, 1, :, :], xt_d[:, 0, 1, :, :])
            nc.gpsimd.dma_start(xt_pre[1][:], xt_d[:, 1, :, :, :])

            # HAM warm-up: the PE clock idles at 1.2 GHz and only reaches
            # 2.4 GHz after ~3.4us of sustained activity. Run dummy matmuls
            # on a zeroed scratch tile while the weight/xt DMAs are in
            # flight so the real matmuls start at full clock.
            scr = wpool.tile([128, 640], F8, tag="scr")
            nc.gpsimd.memset(scr[:, :], 0.0)
            for i in range(N_WARM):
                nc.tensor.matmul(ps2_t[i % 3][:, :], scr[:, 0:128],
                                 scr[:, 128:640], start=True, stop=True)
            w1_sb = w8[:, OFF_W1:OFF_W1 + 1664].rearrange(
                "p (k j m) -> p k j m", k=2, j=2, m=416)
            w2_sb = w8[:, OFF_W2:OFF_W2 + 1216].rearrange(
                "p (k j m) -> p k j m", k=2, j=2, m=304)
            wh1_sb = w8[:, OFF_WH1:OFF_WH1 + 8].rearrange(
                "p (j m) -> p j m", j=2, m=4)
            wh2_sb = w8[0:99, OFF_WH2:OFF_WH2 + 4]
            b1_sb = w8[:, OFF_B1:OFF_B1 + 16].bitcast(F32)      # [128, 4]
            b2_sb = w8[:, OFF_B2:OFF_B2 + 12].bitcast(F32)      # [128, 3]
            eps_sb = w8[:, OFF_EPS:WBYTES].bitcast(F32)         # [128, r/128]

            # Persistent rotating buffers whose zero regions are memset ONCE:
            #  h1b: j=0 -> h1 chunk2 (rewritten each tile); j=1 partitions
            #       0..31 -> h1 chunk3 (rewritten; rows 16..31 zero via zero
            #       weights); j=1 partitions 32..127 -> zero forever.
            #  c2:  99 partitions: [0:44] h2 chunk (rewritten), [44:96] zero
            #       forever, [96:99] (cr, la, const-8) DMA'd each tile.
            NB1 = 6
            NB2 = 12
            h1b_bufs = [wpool.tile([128, 2, NT], F8, tag=f"h1b{i}",
                                   name=f"h1b{i}") for i in range(NB1)]
            c2_bufs = [wpool.tile([99, NT], F8, tag=f"c2{i}",
                                  name=f"c2{i}") for i in range(NB2)]
            # memsets in first-use order (buf i is first read in iteration
            # ~i): each one holds the gpsimd queue ~0.5us, so late buffers'
            # memsets must not delay early buffers'
            for i in range(max(NB1, NB2)):
                if i < NB1:
                    hb = h1b_bufs[i]
                    nc.gpsimd.memset(hb[32:64, 1, :], 0.0)
                    nc.gpsimd.memset(hb[64:128, 1, :], 0.0)
                if i < NB2:
                    cb = c2_bufs[i]
                    nc.gpsimd.memset(cb[32:64, :], 0.0)
                    nc.gpsimd.memset(cb[64:96, :], 0.0)

            utiles = {}     # tile u -> (c1, c2) for the head
            gps = {}        # group g -> (psh, obt)
            gmap = group_layout(n_tiles)
            pending_out = []
            pending_epi = []

            def flush_out(now=None):
                # defer each out descriptor ~6 iterations: the sync engine
                # runs that far ahead of compute, so by the time it reaches
                # the descriptor the epilogue it waits on has completed and
                # it can't head-of-line block the xt descriptors behind it
                while pending_out and (now is None
                                       or pending_out[0][0] <= now - 6):
                    _, osl, obt = pending_out.pop(0)
                    nc.sync.dma_start(out_d[:, osl, :], obt[:])

            def flush_epi(now):
                # Group epilogue, deferred one iteration so the sigmoid sits
                # behind the next tile's fc2 relus in the ACT queue — the
                # head matmuls needing those relus aren't pushed back.
                while pending_epi:
                    g, base, gc, obt = pending_epi.pop(0)
                    psh = psh2[:, g % 2]
                    # psum = 64*(z + b);  pl = sigmoid(z + b) etc.
                    nc.scalar.activation(obt[:, :, 0:2], psh[:, 0:gc, 0:2],
                                         AF.Sigmoid, scale=1.0 / 64.0)
                    nc.vector.tensor_scalar_mul(obt[:, :, 2], psh[:, 0:gc, 2],
                                                1.0 / 64.0)
                    se = spool.tile([128, gc], F32, tag="se")
                    nc.vector.tensor_mul(se[:], obt[:, :, 1],
                                         eps_sb[:, ds(base, gc)])
                    nc.vector.tensor_add(obt[:, :, 3], obt[:, :, 0], se[:])
                    pending_out.append((now, ds(base, gc), obt))

            def emit_head_phase(u):
                """Emit head chunks for the rows of tile u; after the last
                phase of a group, the sampling epilogue + out DMA."""
                g, b, og, base = gmap[u]
                gc = og * RC
                c1, c2 = utiles.pop(u)
                psh = psh2[:, g % 2]
                if b == 0:
                    gps[g] = opool.tile([128, gc, 4], F32, tag="obt",
                                        name="obt")
                obt = gps[g]
                for q in range(RC):
                    c = RC * b + q
                    rsl = ds(q * 128, 128)
                    nc.tensor.matmul(psh[:, c, :], c2[:, rsl], wh2_sb,
                                     start=True, stop=False)
                    nc.tensor.matmul(psh[:, c, :], c1[:, 0, rsl],
                                     wh1_sb[:, 0, :], start=False, stop=False)
                    nc.tensor.matmul(psh[:, c, :], c1[:, 1, rsl],
                                     wh1_sb[:, 1, :], start=False, stop=True)
                if b == og - 1:
                    # psum = 64*(z + b);  pl = sigmoid(z + b) etc.
                    nc.scalar.activation(obt[:, :, 0:2], psh[:, 0:gc, 0:2],
                                         AF.Sigmoid, scale=1.0 / 64.0)
                    nc.vector.tensor_scalar_mul(obt[:, :, 2], psh[:, 0:gc, 2],
                                                1.0 / 64.0)
                    se = spool.tile([128, gc], F32, tag="se")
                    nc.vector.tensor_mul(se[:], obt[:, :, 1],
                                         eps_sb[:, ds(base, gc)])
                    nc.vector.tensor_add(obt[:, :, 3], obt[:, :, 0], se[:])
                    # deferred: emitted on the sync queue next iteration so
                    # the sigmoid-gated descriptor can't head-of-line block
                    # the xt descriptor right behind it
                    pending_out.append((u + 2, ds(base, gc), obt))
                    del gps[g]

            def emit_fc2(t, h1a, h1b, c1, c2):
                # fc2: h2T chunks {128, 128, 44}; psum = 64*y2; the m=2
                # chunk goes first so c2's assembly (relu + rwla DMA)
                # finishes earliest
                for m in (2, 0, 1):
                    m0, mw = M2[m]
                    ps2 = ps2_t[m]
                    for k in range(2):
                        rhs = h1a if k == 0 else h1b
                        nc.tensor.matmul(
                            ps2[0:mw, :],
                            w2_sb[:, k, :, ds(m0, mw)],
                            rhs[:, :, :],
                            start=(k == 0),
                            stop=(k == 1),
                            perf_mode=DR,
                        )
                    # relu(64y2/8 + 8b2) on ACT -> 8*h2 in fp8
                    if m < 2:
                        nc.scalar.activation(c1[:, m, :], ps2[0:mw, :],
                                             AF.Relu,
                                             bias=b2_sb[0:mw, m:m + 1],
                                             scale=0.125)
                    else:
                        nc.scalar.activation(c2[0:44, :], ps2[0:mw, :],
                                             AF.Relu,
                                             bias=b2_sb[0:mw, m:m + 1],
                                             scale=0.125)
                utiles[t] = (c1, c2)

            fc1_out = {}    # tile t -> (h1a, h1b) for the lagged fc2
            fc2_in = {}     # tile t -> (c1, c2)

            for t in range(n_tiles + 2):
                if t > 0:
                    flush_out(t)
                if t < n_tiles:
                    if t < 2:
                        xt_t = xt_pre[t]
                    else:
                        xt_t = xpool.tile([128, 2, 2, NT], F8, tag="xt")
                        nc.sync.dma_start(xt_t[:], xt_d[:, t, :, :, :])
                    h1b = h1b_bufs[t % NB1]
                    c2 = c2_bufs[t % NB2]
                    nc.sync.dma_start(c2[96:99, :], rwla_d[:, ts(t, NT)])

                    # fc1: h1T chunks {128,128,128,32}; psum = 8*y1. Tile 0
                    # runs all k=0 matmuls first (they only need the first
                    # halves of the w1/xt DMAs).
                    h1a = h1apool.tile([128, 2, NT], F8, tag="h1a")
                    korder = ([(k, c) for k in range(2) for c in range(4)]
                              if t == 0 else
                              [(k, c) for c in range(4) for k in range(2)])
                    for k, c in korder:
                        m0, mw = M1[c]
                        ps = ps1_t[c]
                        nc.tensor.matmul(
                            ps[0:mw, :],
                            w1_sb[:, k, :, ds(m0, mw)],
                            xt_t[:, k, :, :],
                            start=(k == 0),
                            stop=(k == 1),
                            perf_mode=DR,
                        )
                        if k != 1:
                            continue
                        # relu((8y1) + 8b1) -> 8*h1 in fp8; the small
                        # chunk goes to ACT to offload the DVE
                        if c < 2:
                            dest = h1a[:, c, :]
                        elif c == 2:
                            dest = h1b[:, 0, :]
                        else:
                            dest = h1b[0:32, 1, :]
                        if c < 3:
                            nc.vector.tensor_scalar(
                                dest, ps[0:mw, :], b1_sb[0:mw, c:c + 1], 0.0,
                                ALU.add, ALU.max
                            )
                        else:
                            nc.scalar.activation(
                                dest, ps[0:mw, :], AF.Relu,
                                bias=b1_sb[0:mw, c:c + 1])
                    fc1_out[t] = (h1a, h1b)
                    fc2_in[t] = (c1pool.tile([128, 2, NT], F8, tag="c1",
                                             name="c1"), c2)

                # head of tile t-2, interleaved between fc1(t) and fc2(t-1)
                if t >= 2:
                    emit_head_phase(t - 2)

                if 1 <= t <= n_tiles:
                    h1a_p, h1b_p = fc1_out.pop(t - 1)
                    c1_p, c2_p = fc2_in.pop(t - 1)
                    emit_fc2(t - 1, h1a_p, h1b_p, c1_p, c2_p)
            flush_out()

    nc.compile()
    return nc


def host_prep(frame, reward, last_action, eps, W1, b1, W2, b2, Wp, bp, Wb, bb,
              rows=R, n_cores=N_CORES):
    """Shard + lay out inputs for the device program. Returns in_maps."""
    frame = np.asarray(frame, np.float32).reshape(TB, OBS)
    reward = np.asarray(reward, np.float32).reshape(TB)
    la = np.asarray(last_action).reshape(TB).astype(np.float32)
    eps = np.asarray(eps, np.float32).reshape(TB)
    n_tiles = rows // NT

    W1 = np.asarray(W1, np.float32)
    W2 = np.asarray(W2, np.float32)
    b1 = np.asarray(b1, np.float32)
    b2 = np.asarray(b2, np.float32)
    Wp = np.asarray(Wp, np.float32)
    bp = np.asarray(bp, np.float32)
    Wb = np.asarray(Wb, np.float32)
    bb = np.asarray(bb, np.float32)

    # frame features f are split as f = 256k + 128j + ki
    frame_q = frame.astype(E4)          # one pass over the big tensor
    W1p = np.zeros((416, 512), np.float32)
    W1p[0:400] = 8.0 * W1
    w1_h = np.ascontiguousarray(
        W1p.T.reshape(2, 2, 128, 416).transpose(2, 0, 1, 3)).astype(E4)
    W2p = np.zeros((304, 512), np.float32)
    W2p[0:300, 0:400] = 8.0 * W2
    w2_h = np.ascontiguousarray(
        W2p.T.reshape(2, 2, 128, 304).transpose(2, 0, 1, 3)).astype(E4)

    # head weights, row-major heads: columns (mu, sigma, baseline, pad);
    # core rows: 0..255 (c1: f = 128j + ki), then c2 rows {0..43: h2
    # 256..299, 44..95: zero, 96: cr, 97: la, 98: const-8 bias row}
    Wh = np.concatenate([Wp, Wb], axis=0)           # [3, 302]
    bh = np.array([bp[0], bp[1], bb[0]], np.float32)
    wh1_h = np.zeros((128, 2, 4), np.float32)
    wh1_h[:, :, 0:3] = (8.0 * Wh[:, 0:256]).T.reshape(2, 128, 3).transpose(
        1, 0, 2)
    wh1_h = wh1_h.astype(E4)
    wh2_h = np.zeros((128, 4), np.float32)
    wh2_h[0:44, 0:3] = 8.0 * Wh[:, 256:300].T
    wh2_h[96:98, 0:3] = 8.0 * Wh[:, 300:302].T
    wh2_h[98, 0:3] = 8.0 * bh
    wh2_h = wh2_h.astype(E4)

    b1s = np.zeros(512, np.float32)
    b1s[0:400] = 8.0 * b1
    b1_h = np.ascontiguousarray(b1s.reshape(4, 128).T)
    b2s = np.zeros(384, np.float32)
    b2s[0:300] = 8.0 * b2
    b2_h = np.ascontiguousarray(b2s.reshape(3, 128).T)

    # fused constant blob (bytes), shared across cores except eps
    wbuf = np.zeros((128, WBYTES), np.uint8)
    wbuf[:, OFF_W1:OFF_W1 + 1664] = w1_h.reshape(128, 1664).view(np.uint8)
    wbuf[:, OFF_W2:OFF_W2 + 1216] = w2_h.reshape(128, 1216).view(np.uint8)
    wbuf[:, OFF_WH1:OFF_WH1 + 8] = wh1_h.reshape(128, 8).view(np.uint8)
    wbuf[:, OFF_WH2:OFF_WH2 + 4] = wh2_h.view(np.uint8)
    wbuf[:, OFF_B1:OFF_B1 + 16] = b1_h.view(np.uint8)
    wbuf[:, OFF_B2:OFF_B2 + 12] = b2_h.view(np.uint8)

    cr8 = (8.0 * np.clip(reward, -1.0, 1.0)).astype(E4)
    la8 = (8.0 * la).astype(E4)
    ones8 = np.full(TB, 8.0, np.float32).astype(E4)

    in_maps = []
    for c in range(n_cores):
        sl = slice(c * rows, (c + 1) * rows)
        xt = np.ascontiguousarray(
            frame_q[sl].T.reshape(2, 2, 128, n_tiles, NT)
            .transpose(2, 3, 0, 1, 4))
        rwla = np.stack([cr8[sl], la8[sl], ones8[sl]], axis=0)
        # eps row r lives at [r % 128, r // 128]
        eps_c = np.ascontiguousarray(eps[sl].reshape(rows // 128, 128).T)
        wb = wbuf.copy()
        wb[:, OFF_EPS:WBYTES] = eps_c.view(np.uint8)
        in_maps.append({
            "xt": xt,
            "rwla": rwla,
            "w8": wb.view(E4),
        })
    return in_maps


def assemble_out(per_core_outs):
    """[128, R//128, 4] per core (row r at [r%128, r//128]) -> [T, B, 4]."""
    outs = []
    for o in per_core_outs:
        o = np.asarray(o)
        outs.append(o.transpose(1, 0, 2).reshape(-1, B, 4))
    return np.ascontiguousarray(
        np.concatenate(outs, axis=0).astype(np.float32))


_NC_CACHE = {}


def kernel(**inputs) -> np.ndarray:
    in_maps = host_prep(**inputs)
    if R not in _NC_CACHE:
        _NC_CACHE[R] = build_bass(R)
    nc = _NC_CACHE[R]
    res = run_bass_kernel_spmd(nc, in_maps, core_ids=list(range(N_CORES)))
    return assemble_out([res.results[c]["out"] for c in range(N_CORES)])



# revision 39
# speedup vs baseline: 1.1319x; 1.0142x over previous
"""Trainium2 Bass kernel for nn_AutoPruneNet (MLP policy/baseline heads + sampling).

Math (per row r of TB = T*B rows):
    h1 = relu(x @ W1.T + b1)            x: [512], h1: [400]
    h2 = relu(h1 @ W2.T + b2)           h2: [300]
    core = [h2, clip(reward,-1,1), last_action]   [302]
    pl = sigmoid(core @ Wp.T + bp)      [2]  (mu, sigma)
    baseline = core @ Wb.T + bb         [1]
    action = pl0 + pl1 * eps
    out[r] = [pl0, pl1, baseline, action]

Distribution: pure data parallel, TB rows split contiguously across 8 cores
(16384 rows each); weights replicated.

Precision: fp8(e4m3) activations + weights with DoubleRow matmuls (2 fp8
weights per PE cell -> K=256 per stream), roughly halving PE streams vs bf16.
Weights are scaled x8 on host so they sit in e4m3's normal range; the scale
compounds through the layers (psum1 = 8*y1, psum2 = 64*y2, psum_h = 64*z) and
is divided back out for free via the ACT engine's input `scale` operand.
Activations are stored as 8*h in fp8.

Layout: fc1/fc2 run feature-major [feature, row] (contraction on partitions,
zero-padded to 512 where needed — K padding costs no PE time, stream cost
depends only on N=512). The HEAD runs row-major: lhsT = core slice
[K, 128 rows] (stationary), rhs = head weights [K, 4] (moving), so the head
psum is [128 rows, (mu,sigma,base,pad)] and the whole sampling epilogue is a
handful of partition-parallel [128, 16, *] ops per 4-tile group instead of
one-partition [1,512] ops. Head biases ride as a constant-8.0 row appended to
the rwla DMA (core row 98) with 8*b head-weight entries.

Schedule: fc2 of tile t-1 is emitted after fc1 of tile t (so fc2 never waits
on the same tile's DVE relus); head phase of tile u runs at iteration u+2.
All constant inputs (weights/biases/eps) are fused into one [128, 3440]-byte
DMA (DMA preamble is packet-rate bound, ~1 packet per partition per
instruction).
"""
import sys
import types

import numpy as np
import ml_dtypes

import concourse.bacc as bacc
import concourse.bass as bass
import concourse.mybir as mybir
import concourse.tile as tile
from concourse.bass import ds, ts
from concourse.bass_utils import run_bass_kernel_spmd


def _install_ntff_hook_shim():
    """Provide the optional antenv.axon_hooks module if the image lacks it,
    so a BASS_TRACE env var in the caller can't crash run_bass_kernel_spmd.
    Registers the real NTFF profile hook when the axon .so supports it."""
    try:
        import antenv.axon_hooks  # noqa: F401
        return
    except Exception:
        pass
    try:
        import antenv
    except Exception:
        return
    mod = types.ModuleType("antenv.axon_hooks")
    state = {"hook": None}
    mod.set_axon_ntff_profile_hook = lambda h: state.__setitem__("hook", h)
    mod.get_axon_ntff_profile_hook = lambda: state["hook"]
    sys.modules["antenv.axon_hooks"] = mod
    antenv.axon_hooks = mod
    try:
        from trn_agent_boot.trn_boot import _ntff_profile_via_ctypes
        mod.set_axon_ntff_profile_hook(
            _ntff_profile_via_ctypes('/opt/axon/libaxon_pjrt.so'))
    except Exception:
        pass


_install_ntff_hook_shim()

E4 = ml_dtypes.float8_e4m3fn

N_CORES = 8
T, B, OBS = 64, 2048, 512
H1, H2 = 400, 300
TB = T * B
R = TB // N_CORES       # rows per core
NT = 512                # rows per row-tile (matmul moving dim)
OG = 4                  # row-tiles per output group (tail groups taper 2,1,1)
RC = NT // 128          # 128-row head chunks per tile (4)
GC = OG * RC            # head chunks per full group (16)
N_WARM = 6              # HAM warm-up matmuls issued during the DMA preamble


def group_layout(n_tiles):
    """Group sizes: full OG-sized groups, tapered tail (2,1,1) so the final
    epilogue chain after the last matmul is short. Returns per-tile
    (group, pos_in_group, group_size, base_chunk)."""
    assert n_tiles % OG == 0 and n_tiles >= 2 * OG
    sizes = [OG] * (n_tiles // OG - 1) + [2, 2]
    per_tile = []
    base = 0
    for g, og in enumerate(sizes):
        for b in range(og):
            per_tile.append((g, b, og, base))
        base += og * RC
    return per_tile

F32 = mybir.dt.float32
F8 = mybir.dt.float8e4
AF = mybir.ActivationFunctionType
ALU = mybir.AluOpType
DR = mybir.MatmulPerfMode.DoubleRow

# fused constant-blob byte offsets (per partition); must match host_prep
OFF_W1 = 0          # [2, 2, 416] fp8
OFF_W2 = 1664       # [2, 2, 304] fp8
OFF_WH1 = 2880      # [2, 4] fp8
OFF_WH2 = 2888      # [4] fp8 (partitions 0..98 used)
OFF_B1 = 2896       # [4] f32
OFF_B2 = 2912       # [3] f32
OFF_EPS = 2928      # [rows//128] f32
WBYTES = 3440

# fc1 output (h1) chunking: {128,128,128,32}; last chunk is 16 real rows of
# h1 plus 16 zero-pad rows (weights zero) so the once-memset zero region of
# h1b's j=1 half starts at partition 32.
M1 = [(0, 128), (128, 128), (256, 128), (384, 32)]
# fc2 output (h2) chunking: {128, 128, 44}
M2 = [(0, 128), (128, 128), (256, 44)]


def build_bass(rows: int):
    """Build the per-core Bass program for `rows` rows (rows % (NT*OG) == 0)."""
    assert rows % (NT * OG) == 0
    assert rows // 128 == (WBYTES - OFF_EPS) // 4
    n_tiles = rows // NT

    nc = bacc.Bacc("TRN2", target_bir_lowering=False, debug=False)

    xt_d = nc.dram_tensor("xt", [128, n_tiles, 2, 2, NT], F8,
                          kind="ExternalInput")
    rwla_d = nc.dram_tensor("rwla", [3, rows], F8, kind="ExternalInput")
    w8_d = nc.dram_tensor("w8", [128, WBYTES], F8, kind="ExternalInput")
    out_d = nc.dram_tensor("out", [128, rows // 128, 4], F32,
                           kind="ExternalOutput")

    with tile.TileContext(nc) as tc:
        with (
            tc.tile_pool(name="w", bufs=1) as wpool,
            tc.tile_pool(name="x", bufs=8) as xpool,
            tc.tile_pool(name="h1a", bufs=6) as h1apool,
            tc.tile_pool(name="c1", bufs=8) as c1pool,
            tc.tile_pool(name="ot", bufs=4) as opool,
            tc.tile_pool(name="s", bufs=4) as spool,
            tc.tile_pool(name="ps1", bufs=1, space="PSUM") as ppool1,
            tc.tile_pool(name="ps2", bufs=1, space="PSUM") as ppool2,
            tc.tile_pool(name="ps3", bufs=1, space="PSUM") as ppool3,
        ):
            w8 = wpool.tile([128, WBYTES], F8, tag="w8")
            # w1 k=0 slice first: the first fc1 matmul needs only it + the
            # k=0 half of xt(0)
            nc.scalar.dma_start(w8[:, 0:832], w8_d[:, 0:832])
            nc.scalar.dma_start(w8[:, 832:OFF_W2], w8_d[:, 832:OFF_W2])
            nc.scalar.dma_start(w8[:, OFF_W2:], w8_d[:, OFF_W2:])

            # Fixed-bank PSUM tiles: one bank per fc1 chunk (4) / fc2 chunk
            # (3) / head group (1). Pool-rotated psum tiles hand the
            # just-freed bank to the next chunk, making every matmul WAR-wait
            # on the relu that read that bank ~750ns earlier; a fixed
            # bank-per-chunk gives a full tile (~3.4us) of slack instead.
            ps1_t = [ppool1.tile([128, NT], F32, tag=f"ps1_{i}",
                                 name=f"ps1_{i}") for i in range(4)]
            ps2_t = [ppool2.tile([128, NT], F32, tag=f"ps2_{i}",
                                 name=f"ps2_{i}") for i in range(3)]
            # two half-bank head-psum slots in one bank, alternating per group
            psh2 = ppool3.tile([128, 2, GC, 4], F32, tag="ps3", name="psh2")

            # Prefetch the first two x tiles before anything else queues:
            # xt(0) halves on the sync queue (the k=0 matmuls start after the
            # first half), xt(1) in parallel on the gpsimd queue (one queue
            # moves ~107 GB/s, not enough for both during ramp). These must
            # precede the zero-region memsets below: each memset occupies the
            # gpsimd queue for ~0.5us.
            xt_pre = [xpool.tile([128, 2, 2, NT], F8, tag="xt", name="xt_pre")
                      for _ in range(2)]
            nc.sync.dma_start(xt_pre[0][:, 0, :, :], xt_d[:, 0, 0, :, :])
            nc.sync.dma_start(xt_pre[0][:, 1, :, :], xt_d[:, 0, 1, :, :])
            nc.gpsimd.dma_start(xt_pre[1][:], xt_d[:, 1, :, :, :])

            # HAM warm-up: the PE clock idles at 1.2 GHz and only reaches
            # 2.4 GHz after ~3.4us of sustained activity. Run dummy matmuls
            # on a zeroed scratch tile while the weight/xt DMAs are in
            # flight so the real matmuls start at full clock.
            scr = wpool.tile([128, 640], F8, tag="scr")
            nc.gpsimd.memset(scr[:, :], 0.0)
            for i in range(N_WARM):
                nc.tensor.matmul(ps2_t[i % 3][:, :], scr[:, 0:128],
                                 scr[:, 128:640], start=True, stop=True)
            w1_sb = w8[:, OFF_W1:OFF_W1 + 1664].rearrange(
                "p (k j m) -> p k j m", k=2, j=2, m=416)
            w2_sb = w8[:, OFF_W2:OFF_W2 + 1216].rearrange(
                "p (k j m) -> p k j m", k=2, j=2, m=304)
            wh1_sb = w8[:, OFF_WH1:OFF_WH1 + 8].rearrange(
                "p (j m) -> p j m", j=2, m=4)
            wh2_sb = w8[0:99, OFF_WH2:OFF_WH2 + 4]
            b1_sb = w8[:, OFF_B1:OFF_B1 + 16].bitcast(F32)      # [128, 4]
            b2_sb = w8[:, OFF_B2:OFF_B2 + 12].bitcast(F32)      # [128, 3]
            eps_sb = w8[:, OFF_EPS:WBYTES].bitcast(F32)         # [128, r/128]

            # Persistent rotating buffers whose zero regions are memset ONCE:
            #  h1b: j=0 -> h1 chunk2 (rewritten each tile); j=1 partitions
            #       0..31 -> h1 chunk3 (rewritten; rows 16..31 zero via zero
            #       weights); j=1 partitions 32..127 -> zero forever.
            #  c2:  99 partitions: [0:44] h2 chunk (rewritten), [44:96] zero
            #       forever, [96:99] (cr, la, const-8) DMA'd each tile.
            NB1 = 6
            NB2 = 12
            h1b_bufs = [wpool.tile([128, 2, NT], F8, tag=f"h1b{i}",
                                   name=f"h1b{i}") for i in range(NB1)]
            c2_bufs = [wpool.tile([99, NT], F8, tag=f"c2{i}",
                                  name=f"c2{i}") for i in range(NB2)]
            # memsets in first-use order (buf i is first read in iteration
            # ~i): each one holds the gpsimd queue ~0.5us, so late buffers'
            # memsets must not delay early buffers'
            for i in range(max(NB1, NB2)):
                if i < NB1:
                    hb = h1b_bufs[i]
                    nc.gpsimd.memset(hb[32:64, 1, :], 0.0)
                    nc.gpsimd.memset(hb[64:128, 1, :], 0.0)
                if i < NB2:
                    cb = c2_bufs[i]
                    nc.gpsimd.memset(cb[32:64, :], 0.0)
                    nc.gpsimd.memset(cb[64:96, :], 0.0)

            utiles = {}     # tile u -> (c1, c2) for the head
            gps = {}        # group g -> (psh, obt)
            gmap = group_layout(n_tiles)
            pending_out = []
            pending_epi = []

            def flush_out(now=None):
                # defer each out descriptor ~6 iterations: the sync engine
                # runs that far ahead of compute, so by the time it reaches
                # the descriptor the epilogue it waits on has completed and
                # it can't head-of-line block the xt descriptors behind it
                while pending_out and (now is None
                                       or pending_out[0][0] <= now - 6):
                    _, osl, obt = pending_out.pop(0)
                    nc.sync.dma_start(out_d[:, osl, :], obt[:])

            def flush_epi(now):
                # Group epilogue, deferred one iteration so the sigmoid sits
                # behind the next tile's fc2 relus in the ACT queue — the
                # head matmuls needing those relus aren't pushed back.
                while pending_epi:
                    g, base, gc, obt = pending_epi.pop(0)
                    psh = psh2[:, g % 2]
                    # psum = 64*(z + b);  pl = sigmoid(z + b) etc.
                    nc.scalar.activation(obt[:, :, 0:2], psh[:, 0:gc, 0:2],
                                         AF.Sigmoid, scale=1.0 / 64.0)
                    nc.vector.tensor_scalar_mul(obt[:, :, 2], psh[:, 0:gc, 2],
                                                1.0 / 64.0)
                    se = spool.tile([128, gc], F32, tag="se")
                    nc.vector.tensor_mul(se[:], obt[:, :, 1],
                                         eps_sb[:, ds(base, gc)])
                    nc.vector.tensor_add(obt[:, :, 3], obt[:, :, 0], se[:])
                    pending_out.append((now, ds(base, gc), obt))

            def emit_head_phase(u):
                """Emit head chunks for the rows of tile u; after the last
                phase of a group, the sampling epilogue + out DMA."""
                g, b, og, base = gmap[u]
                gc = og * RC
                c1, c2 = utiles.pop(u)
                psh = psh2[:, g % 2]
                if b == 0:
                    gps[g] = opool.tile([128, gc, 4], F32, tag="obt",
                                        name="obt")
                obt = gps[g]
                for q in range(RC):
                    c = RC * b + q
                    rsl = ds(q * 128, 128)
                    nc.tensor.matmul(psh[:, c, :], c2[:, rsl], wh2_sb,
                                     start=True, stop=False)
                    nc.tensor.matmul(psh[:, c, :], c1[:, 0, rsl],
                                     wh1_sb[:, 0, :], start=False, stop=False)
                    nc.tensor.matmul(psh[:, c, :], c1[:, 1, rsl],
                                     wh1_sb[:, 1, :], start=False, stop=True)
                if b == og - 1:
                    pending_epi.append((g, base, gc, obt))
                    del gps[g]

            def emit_fc2(t, h1a, h1b, c1, c2):
                # fc2: h2T chunks {128, 128, 44}; psum = 64*y2; the m=2
                # chunk goes first so c2's assembly (relu + rwla DMA)
                # finishes earliest
                for m in (2, 0, 1):
                    m0, mw = M2[m]
                    ps2 = ps2_t[m]
                    for k in range(2):
                        rhs = h1a if k == 0 else h1b
                        nc.tensor.matmul(
                            ps2[0:mw, :],
                            w2_sb[:, k, :, ds(m0, mw)],
                            rhs[:, :, :],
                            start=(k == 0),
                            stop=(k == 1),
                            perf_mode=DR,
                        )
                    # relu(64y2/8 + 8b2) on ACT -> 8*h2 in fp8
                    if m < 2:
                        nc.scalar.activation(c1[:, m, :], ps2[0:mw, :],
                                             AF.Relu,
                                             bias=b2_sb[0:mw, m:m + 1],
                                             scale=0.125)
                    else:
                        nc.scalar.activation(c2[0:44, :], ps2[0:mw, :],
                                             AF.Relu,
                                             bias=b2_sb[0:mw, m:m + 1],
                                             scale=0.125)
                utiles[t] = (c1, c2)

            fc1_out = {}    # tile t -> (h1a, h1b) for the lagged fc2
            fc2_in = {}     # tile t -> (c1, c2)

            for t in range(n_tiles + 2):
                if t > 0:
                    flush_out(t)
                if t < n_tiles:
                    if t < 2:
                        xt_t = xt_pre[t]
                    else:
                        xt_t = xpool.tile([128, 2, 2, NT], F8, tag="xt")
                        nc.sync.dma_start(xt_t[:], xt_d[:, t, :, :, :])
                    h1b = h1b_bufs[t % NB1]
                    c2 = c2_bufs[t % NB2]
                    nc.sync.dma_start(c2[96:99, :], rwla_d[:, ts(t, NT)])

                    # fc1: h1T chunks {128,128,128,32}; psum = 8*y1. Tile 0
                    # runs all k=0 matmuls first (they only need the first
                    # halves of the w1/xt DMAs).
                    h1a = h1apool.tile([128, 2, NT], F8, tag="h1a")
                    korder = ([(k, c) for k in range(2) for c in range(4)]
                              if t == 0 else
                              [(k, c) for c in range(4) for k in range(2)])
                    for k, c in korder:
                        m0, mw = M1[c]
                        ps = ps1_t[c]
                        nc.tensor.matmul(
                            ps[0:mw, :],
                            w1_sb[:, k, :, ds(m0, mw)],
                            xt_t[:, k, :, :],
                            start=(k == 0),
                            stop=(k == 1),
                            perf_mode=DR,
                        )
                        if k != 1:
                            continue
                        # relu((8y1) + 8b1) -> 8*h1 in fp8; the small
                        # chunk goes to ACT to offload the DVE
                        if c < 2:
                            dest = h1a[:, c, :]
                        elif c == 2:
                            dest = h1b[:, 0, :]
                        else:
                            dest = h1b[0:32, 1, :]
                        if c < 3:
                            nc.vector.tensor_scalar(
                                dest, ps[0:mw, :], b1_sb[0:mw, c:c + 1], 0.0,
                                ALU.add, ALU.max
                            )
                        else:
                            nc.scalar.activation(
                                dest, ps[0:mw, :], AF.Relu,
                                bias=b1_sb[0:mw, c:c + 1])
                    fc1_out[t] = (h1a, h1b)
                    fc2_in[t] = (c1pool.tile([128, 2, NT], F8, tag="c1",
                                             name="c1"), c2)

                # head of tile t-2, interleaved between fc1(t) and fc2(t-1)
                if t >= 2:
                    emit_head_phase(t - 2)

                if 1 <= t <= n_tiles:
                    h1a_p, h1b_p = fc1_out.pop(t - 1)
                    c1_p, c2_p = fc2_in.pop(t - 1)
                    emit_fc2(t - 1, h1a_p, h1b_p, c1_p, c2_p)
                flush_epi(t)
            flush_out()

    nc.compile()
    return nc


def host_prep(frame, reward, last_action, eps, W1, b1, W2, b2, Wp, bp, Wb, bb,
              rows=R, n_cores=N_CORES):
    """Shard + lay out inputs for the device program. Returns in_maps."""
    frame = np.asarray(frame, np.float32).reshape(TB, OBS)
    reward = np.asarray(reward, np.float32).reshape(TB)
    la = np.asarray(last_action).reshape(TB).astype(np.float32)
    eps = np.asarray(eps, np.float32).reshape(TB)
    n_tiles = rows // NT

    W1 = np.asarray(W1, np.float32)
    W2 = np.asarray(W2, np.float32)
    b1 = np.asarray(b1, np.float32)
    b2 = np.asarray(b2, np.float32)
    Wp = np.asarray(Wp, np.float32)
    bp = np.asarray(bp, np.float32)
    Wb = np.asarray(Wb, np.float32)
    bb = np.asarray(bb, np.float32)

    # frame features f are split as f = 256k + 128j + ki
    frame_q = frame.astype(E4)          # one pass over the big tensor
    W1p = np.zeros((416, 512), np.float32)
    W1p[0:400] = 8.0 * W1
    w1_h = np.ascontiguousarray(
        W1p.T.reshape(2, 2, 128, 416).transpose(2, 0, 1, 3)).astype(E4)
    W2p = np.zeros((304, 512), np.float32)
    W2p[0:300, 0:400] = 8.0 * W2
    w2_h = np.ascontiguousarray(
        W2p.T.reshape(2, 2, 128, 304).transpose(2, 0, 1, 3)).astype(E4)

    # head weights, row-major heads: columns (mu, sigma, baseline, pad);
    # core rows: 0..255 (c1: f = 128j + ki), then c2 rows {0..43: h2
    # 256..299, 44..95: zero, 96: cr, 97: la, 98: const-8 bias row}
    Wh = np.concatenate([Wp, Wb], axis=0)           # [3, 302]
    bh = np.array([bp[0], bp[1], bb[0]], np.float32)
    wh1_h = np.zeros((128, 2, 4), np.float32)
    wh1_h[:, :, 0:3] = (8.0 * Wh[:, 0:256]).T.reshape(2, 128, 3).transpose(
        1, 0, 2)
    wh1_h = wh1_h.astype(E4)
    wh2_h = np.zeros((128, 4), np.float32)
    wh2_h[0:44, 0:3] = 8.0 * Wh[:, 256:300].T
    wh2_h[96:98, 0:3] = 8.0 * Wh[:, 300:302].T
    wh2_h[98, 0:3] = 8.0 * bh
    wh2_h = wh2_h.astype(E4)

    b1s = np.zeros(512, np.float32)
    b1s[0:400] = 8.0 * b1
    b1_h = np.ascontiguousarray(b1s.reshape(4, 128).T)
    b2s = np.zeros(384, np.float32)
    b2s[0:300] = 8.0 * b2
    b2_h = np.ascontiguousarray(b2s.reshape(3, 128).T)

    # fused constant blob (bytes), shared across cores except eps
    wbuf = np.zeros((128, WBYTES), np.uint8)
    wbuf[:, OFF_W1:OFF_W1 + 1664] = w1_h.reshape(128, 1664).view(np.uint8)
    wbuf[:, OFF_W2:OFF_W2 + 1216] = w2_h.reshape(128, 1216).view(np.uint8)
    wbuf[:, OFF_WH1:OFF_WH1 + 8] = wh1_h.reshape(128, 8).view(np.uint8)
    wbuf[:, OFF_WH2:OFF_WH2 + 4] = wh2_h.view(np.uint8)
    wbuf[:, OFF_B1:OFF_B1 + 16] = b1_h.view(np.uint8)
    wbuf[:, OFF_B2:OFF_B2 + 12] = b2_h.view(np.uint8)

    cr8 = (8.0 * np.clip(reward, -1.0, 1.0)).astype(E4)
    la8 = (8.0 * la).astype(E4)
    ones8 = np.full(TB, 8.0, np.float32).astype(E4)

    in_maps = []
    for c in range(n_cores):
        sl = slice(c * rows, (c + 1) * rows)
        xt = np.ascontiguousarray(
            frame_q[sl].T.reshape(2, 2, 128, n_tiles, NT)
            .transpose(2, 3, 0, 1, 4))
        rwla = np.stack([cr8[sl], la8[sl], ones8[sl]], axis=0)
        # eps row r lives at [r % 128, r // 128]
        eps_c = np.ascontiguousarray(eps[sl].reshape(rows // 128, 128).T)
        wb = wbuf.copy()
        wb[:, OFF_EPS:WBYTES] = eps_c.view(np.uint8)
        in_maps.append({
            "xt": xt,
            "rwla": rwla,
            "w8": wb.view(E4),
        })
    return in_maps


def assemble_out(per_core_outs):
    """[128, R//128, 4] per core (row r at [r%128, r//128]) -> [T, B, 4]."""
    outs = []
    for o in per_core_outs:
        o = np.asarray(o)
        outs.append(o.transpose(1, 0, 2).reshape(-1, B, 4))
    return np.ascontiguousarray(
        np.concatenate(outs, axis=0).astype(np.float32))


_NC_CACHE = {}


def kernel(**inputs) -> np.ndarray:
    in_maps = host_prep(**inputs)
    if R not in _NC_CACHE:
        _NC_CACHE[R] = build_bass(R)
    nc = _NC_CACHE[R]
    res = run_bass_kernel_spmd(nc, in_maps, core_ids=list(range(N_CORES)))
    return assemble_out([res.results[c]["out"] for c in range(N_CORES)])



# revision 41
# speedup vs baseline: 1.1331x; 1.0011x over previous
"""Trainium2 Bass kernel for nn_AutoPruneNet (MLP policy/baseline heads + sampling).

Math (per row r of TB = T*B rows):
    h1 = relu(x @ W1.T + b1)            x: [512], h1: [400]
    h2 = relu(h1 @ W2.T + b2)           h2: [300]
    core = [h2, clip(reward,-1,1), last_action]   [302]
    pl = sigmoid(core @ Wp.T + bp)      [2]  (mu, sigma)
    baseline = core @ Wb.T + bb         [1]
    action = pl0 + pl1 * eps
    out[r] = [pl0, pl1, baseline, action]

Distribution: pure data parallel, TB rows split contiguously across 8 cores
(16384 rows each); weights replicated.

Precision: fp8(e4m3) activations + weights with DoubleRow matmuls (2 fp8
weights per PE cell -> K=256 per stream), roughly halving PE streams vs bf16.
Weights are scaled x8 on host so they sit in e4m3's normal range; the scale
compounds through the layers (psum1 = 8*y1, psum2 = 64*y2, psum_h = 64*z) and
is divided back out for free via the ACT engine's input `scale` operand.
Activations are stored as 8*h in fp8.

Layout: fc1/fc2 run feature-major [feature, row] (contraction on partitions,
zero-padded to 512 where needed — K padding costs no PE time, stream cost
depends only on N=512). The HEAD runs row-major: lhsT = core slice
[K, 128 rows] (stationary), rhs = head weights [K, 4] (moving), so the head
psum is [128 rows, (mu,sigma,base,pad)] and the whole sampling epilogue is a
handful of partition-parallel [128, 16, *] ops per 4-tile group instead of
one-partition [1,512] ops. Head biases ride as a constant-8.0 row appended to
the rwla DMA (core row 98) with 8*b head-weight entries.

Schedule: fc2 of tile t-1 is emitted after fc1 of tile t (so fc2 never waits
on the same tile's DVE relus); head phase of tile u runs at iteration u+2.
All constant inputs (weights/biases/eps) are fused into one [128, 3440]-byte
DMA (DMA preamble is packet-rate bound, ~1 packet per partition per
instruction).
"""
import sys
import types

import numpy as np
import ml_dtypes

import concourse.bacc as bacc
import concourse.bass as bass
import concourse.mybir as mybir
import concourse.tile as tile
from concourse.bass import ds, ts
from concourse.bass_utils import run_bass_kernel_spmd


def _install_ntff_hook_shim():
    """Provide the optional antenv.axon_hooks module if the image lacks it,
    so a BASS_TRACE env var in the caller can't crash run_bass_kernel_spmd.
    Registers the real NTFF profile hook when the axon .so supports it."""
    try:
        import antenv.axon_hooks  # noqa: F401
        return
    except Exception:
        pass
    try:
        import antenv
    except Exception:
        return
    mod = types.ModuleType("antenv.axon_hooks")
    state = {"hook": None}
    mod.set_axon_ntff_profile_hook = lambda h: state.__setitem__("hook", h)
    mod.get_axon_ntff_profile_hook = lambda: state["hook"]
    sys.modules["antenv.axon_hooks"] = mod
    antenv.axon_hooks = mod
    try:
        from trn_agent_boot.trn_boot import _ntff_profile_via_ctypes
        mod.set_axon_ntff_profile_hook(
            _ntff_profile_via_ctypes('/opt/axon/libaxon_pjrt.so'))
    except Exception:
        pass


_install_ntff_hook_shim()

E4 = ml_dtypes.float8_e4m3fn

N_CORES = 8
T, B, OBS = 64, 2048, 512
H1, H2 = 400, 300
TB = T * B
R = TB // N_CORES       # rows per core
NT = 512                # rows per row-tile (matmul moving dim)
OG = 4                  # row-tiles per output group (tail groups taper 2,1,1)
RC = NT // 128          # 128-row head chunks per tile (4)
GC = OG * RC            # head chunks per full group (16)
N_WARM = 6              # HAM warm-up matmuls issued during the DMA preamble


def group_layout(n_tiles):
    """Group sizes: full OG-sized groups, tapered tail (2,1,1) so the final
    epilogue chain after the last matmul is short. Returns per-tile
    (group, pos_in_group, group_size, base_chunk)."""
    assert n_tiles % OG == 0 and n_tiles >= 2 * OG
    sizes = [OG] * (n_tiles // OG - 1) + [2, 2]
    per_tile = []
    base = 0
    for g, og in enumerate(sizes):
        for b in range(og):
            per_tile.append((g, b, og, base))
        base += og * RC
    return per_tile

F32 = mybir.dt.float32
F8 = mybir.dt.float8e4
AF = mybir.ActivationFunctionType
ALU = mybir.AluOpType
DR = mybir.MatmulPerfMode.DoubleRow

# fused constant-blob byte offsets (per partition); must match host_prep
OFF_W1 = 0          # [2, 2, 416] fp8
OFF_W2 = 1664       # [2, 2, 304] fp8
OFF_WH1 = 2880      # [2, 4] fp8
OFF_WH2 = 2888      # [4] fp8 (partitions 0..98 used)
OFF_B1 = 2896       # [4] f32
OFF_B2 = 2912       # [3] f32
OFF_EPS = 2928      # [rows//128] f32
WBYTES = 3440

# fc1 output (h1) chunking: {128,128,128,32}; last chunk is 16 real rows of
# h1 plus 16 zero-pad rows (weights zero) so the once-memset zero region of
# h1b's j=1 half starts at partition 32.
M1 = [(0, 128), (128, 128), (256, 128), (384, 32)]
# fc2 output (h2) chunking: {128, 128, 44}
M2 = [(0, 128), (128, 128), (256, 44)]


def build_bass(rows: int):
    """Build the per-core Bass program for `rows` rows (rows % (NT*OG) == 0)."""
    assert rows % (NT * OG) == 0
    assert rows // 128 == (WBYTES - OFF_EPS) // 4
    n_tiles = rows // NT

    nc = bacc.Bacc("TRN2", target_bir_lowering=False, debug=False)

    xt_d = nc.dram_tensor("xt", [128, n_tiles, 2, 2, NT], F8,
                          kind="ExternalInput")
    rwla_d = nc.dram_tensor("rwla", [3, rows], F8, kind="ExternalInput")
    w8_d = nc.dram_tensor("w8", [128, WBYTES], F8, kind="ExternalInput")
    out_d = nc.dram_tensor("out", [128, rows // 128, 4], F32,
                           kind="ExternalOutput")

    with tile.TileContext(nc) as tc:
        with (
            tc.tile_pool(name="w", bufs=1) as wpool,
            tc.tile_pool(name="x", bufs=8) as xpool,
            tc.tile_pool(name="h1a", bufs=6) as h1apool,
            tc.tile_pool(name="c1", bufs=8) as c1pool,
            tc.tile_pool(name="ot", bufs=4) as opool,
            tc.tile_pool(name="s", bufs=4) as spool,
            tc.tile_pool(name="ps1", bufs=1, space="PSUM") as ppool1,
            tc.tile_pool(name="ps2", bufs=1, space="PSUM") as ppool2,
            tc.tile_pool(name="ps3", bufs=1, space="PSUM") as ppool3,
        ):
            w8 = wpool.tile([128, WBYTES], F8, tag="w8")
            # w1 k=0 slice first: the first fc1 matmul needs only it + the
            # k=0 half of xt(0)
            nc.scalar.dma_start(w8[:, 0:832], w8_d[:, 0:832])
            nc.scalar.dma_start(w8[:, 832:OFF_W2], w8_d[:, 832:OFF_W2])
            nc.scalar.dma_start(w8[:, OFF_W2:], w8_d[:, OFF_W2:])

            # Fixed-bank PSUM tiles: one bank per fc1 chunk (4) / fc2 chunk
            # (3) / head group (1). Pool-rotated psum tiles hand the
            # just-freed bank to the next chunk, making every matmul WAR-wait
            # on the relu that read that bank ~750ns earlier; a fixed
            # bank-per-chunk gives a full tile (~3.4us) of slack instead.
            ps1_t = [ppool1.tile([128, NT], F32, tag=f"ps1_{i}",
                                 name=f"ps1_{i}") for i in range(4)]
            ps2_t = [ppool2.tile([128, NT], F32, tag=f"ps2_{i}",
                                 name=f"ps2_{i}") for i in range(3)]
            # two half-bank head-psum slots in one bank, alternating per group
            psh2 = ppool3.tile([128, 2, GC, 4], F32, tag="ps3", name="psh2")

            # warm-up scratch: memset first so the dummy matmuls can start
            # as early as possible
            scr = wpool.tile([128, 640], F8, tag="scr")
            nc.gpsimd.memset(scr[:, :], 0.0)

            # Prefetch the first two x tiles before anything else queues:
            # xt(0) halves on the sync queue, xt(1) halves in parallel on the
            # gpsimd queue (one queue moves ~107 GB/s, not enough for both
            # during ramp); the k=0 matmuls of each tile start after the
            # first half. These must precede the zero-region memsets below:
            # each memset occupies the gpsimd queue for ~0.5us.
            xt_pre = [xpool.tile([128, 2, 2, NT], F8, tag="xt", name="xt_pre")
                      for _ in range(2)]
            nc.sync.dma_start(xt_pre[0][:, 0, :, :], xt_d[:, 0, 0, :, :])
            nc.sync.dma_start(xt_pre[0][:, 1, :, :], xt_d[:, 0, 1, :, :])
            nc.gpsimd.dma_start(xt_pre[1][:, 0, :, :], xt_d[:, 1, 0, :, :])
            nc.gpsimd.dma_start(xt_pre[1][:, 1, :, :], xt_d[:, 1, 1, :, :])

            # HAM warm-up: the PE clock idles at 1.2 GHz and only reaches
            # 2.4 GHz after ~3.4us of sustained activity. Run dummy matmuls
            # on a zeroed scratch tile while the weight/xt DMAs are in
            # flight so the real matmuls start at full clock.
            for i in range(N_WARM):
                nc.tensor.matmul(ps2_t[i % 3][:, :], scr[:, 0:128],
                                 scr[:, 128:640], start=True, stop=True)
            w1_sb = w8[:, OFF_W1:OFF_W1 + 1664].rearrange(
                "p (k j m) -> p k j m", k=2, j=2, m=416)
            w2_sb = w8[:, OFF_W2:OFF_W2 + 1216].rearrange(
                "p (k j m) -> p k j m", k=2, j=2, m=304)
            wh1_sb = w8[:, OFF_WH1:OFF_WH1 + 8].rearrange(
                "p (j m) -> p j m", j=2, m=4)
            wh2_sb = w8[0:99, OFF_WH2:OFF_WH2 + 4]
            b1_sb = w8[:, OFF_B1:OFF_B1 + 16].bitcast(F32)      # [128, 4]
            b2_sb = w8[:, OFF_B2:OFF_B2 + 12].bitcast(F32)      # [128, 3]
            eps_sb = w8[:, OFF_EPS:WBYTES].bitcast(F32)         # [128, r/128]

            # Persistent rotating buffers whose zero regions are memset ONCE:
            #  h1b: j=0 -> h1 chunk2 (rewritten each tile); j=1 partitions
            #       0..31 -> h1 chunk3 (rewritten; rows 16..31 zero via zero
            #       weights); j=1 partitions 32..127 -> zero forever.
            #  c2:  99 partitions: [0:44] h2 chunk (rewritten), [44:96] zero
            #       forever, [96:99] (cr, la, const-8) DMA'd each tile.
            NB1 = 6
            NB2 = 12
            h1b_bufs = [wpool.tile([128, 2, NT], F8, tag=f"h1b{i}",
                                   name=f"h1b{i}") for i in range(NB1)]
            c2_bufs = [wpool.tile([99, NT], F8, tag=f"c2{i}",
                                  name=f"c2{i}") for i in range(NB2)]
            # memsets in first-use order (buf i is first read in iteration
            # ~i): each one holds the gpsimd queue ~0.5us, so late buffers'
            # memsets must not delay early buffers'
            for i in range(max(NB1, NB2)):
                if i < NB1:
                    hb = h1b_bufs[i]
                    nc.gpsimd.memset(hb[32:64, 1, :], 0.0)
                    nc.gpsimd.memset(hb[64:128, 1, :], 0.0)
                if i < NB2:
                    cb = c2_bufs[i]
                    nc.gpsimd.memset(cb[32:64, :], 0.0)
                    nc.gpsimd.memset(cb[64:96, :], 0.0)

            utiles = {}     # tile u -> (c1, c2) for the head
            gps = {}        # group g -> (psh, obt)
            gmap = group_layout(n_tiles)
            pending_out = []
            pending_epi = []

            def flush_out(now=None):
                # defer each out descriptor ~6 iterations: the sync engine
                # runs that far ahead of compute, so by the time it reaches
                # the descriptor the epilogue it waits on has completed and
                # it can't head-of-line block the xt descriptors behind it
                while pending_out and (now is None
                                       or pending_out[0][0] <= now - 6):
                    _, osl, obt = pending_out.pop(0)
                    nc.sync.dma_start(out_d[:, osl, :], obt[:])

            def flush_epi(now):
                # Group epilogue, deferred one iteration so the sigmoid sits
                # behind the next tile's fc2 relus in the ACT queue — the
                # head matmuls needing those relus aren't pushed back.
                while pending_epi:
                    g, base, gc, obt = pending_epi.pop(0)
                    psh = psh2[:, g % 2]
                    # psum = 64*(z + b);  pl = sigmoid(z + b) etc.
                    nc.scalar.activation(obt[:, :, 0:2], psh[:, 0:gc, 0:2],
                                         AF.Sigmoid, scale=1.0 / 64.0)
                    nc.vector.tensor_scalar_mul(obt[:, :, 2], psh[:, 0:gc, 2],
                                                1.0 / 64.0)
                    se = spool.tile([128, gc], F32, tag="se")
                    nc.vector.tensor_mul(se[:], obt[:, :, 1],
                                         eps_sb[:, ds(base, gc)])
                    nc.vector.tensor_add(obt[:, :, 3], obt[:, :, 0], se[:])
                    pending_out.append((now, ds(base, gc), obt))

            def emit_head_phase(u):
                """Emit head chunks for the rows of tile u; after the last
                phase of a group, the sampling epilogue + out DMA."""
                g, b, og, base = gmap[u]
                gc = og * RC
                c1, c2 = utiles.pop(u)
                psh = psh2[:, g % 2]
                if b == 0:
                    gps[g] = opool.tile([128, gc, 4], F32, tag="obt",
                                        name="obt")
                obt = gps[g]
                for q in range(RC):
                    c = RC * b + q
                    rsl = ds(q * 128, 128)
                    nc.tensor.matmul(psh[:, c, :], c2[:, rsl], wh2_sb,
                                     start=True, stop=False)
                    nc.tensor.matmul(psh[:, c, :], c1[:, 0, rsl],
                                     wh1_sb[:, 0, :], start=False, stop=False)
                    nc.tensor.matmul(psh[:, c, :], c1[:, 1, rsl],
                                     wh1_sb[:, 1, :], start=False, stop=True)
                if b == og - 1:
                    pending_epi.append((g, base, gc, obt))
                    del gps[g]

            def emit_fc2(t, h1a, h1b, c1, c2):
                # fc2: h2T chunks {128, 128, 44}; psum = 64*y2; the m=2
                # chunk goes first so c2's assembly (relu + rwla DMA)
                # finishes earliest
                for m in (2, 0, 1):
                    m0, mw = M2[m]
                    ps2 = ps2_t[m]
                    for k in range(2):
                        rhs = h1a if k == 0 else h1b
                        nc.tensor.matmul(
                            ps2[0:mw, :],
                            w2_sb[:, k, :, ds(m0, mw)],
                            rhs[:, :, :],
                            start=(k == 0),
                            stop=(k == 1),
                            perf_mode=DR,
                        )
                    # relu(64y2/8 + 8b2) on ACT -> 8*h2 in fp8
                    if m < 2:
                        nc.scalar.activation(c1[:, m, :], ps2[0:mw, :],
                                             AF.Relu,
                                             bias=b2_sb[0:mw, m:m + 1],
                                             scale=0.125)
                    else:
                        nc.scalar.activation(c2[0:44, :], ps2[0:mw, :],
                                             AF.Relu,
                                             bias=b2_sb[0:mw, m:m + 1],
                                             scale=0.125)
                utiles[t] = (c1, c2)

            fc1_out = {}    # tile t -> (h1a, h1b) for the lagged fc2
            fc2_in = {}     # tile t -> (c1, c2)

            for t in range(n_tiles + 2):
                if t > 0:
                    flush_out(t)
                if t < n_tiles:
                    if t < 2:
                        xt_t = xt_pre[t]
                    else:
                        xt_t = xpool.tile([128, 2, 2, NT], F8, tag="xt")
                        nc.sync.dma_start(xt_t[:], xt_d[:, t, :, :, :])
                    h1b = h1b_bufs[t % NB1]
                    c2 = c2_bufs[t % NB2]
                    nc.sync.dma_start(c2[96:99, :], rwla_d[:, ts(t, NT)])

                    # fc1: h1T chunks {128,128,128,32}; psum = 8*y1. Tile 0
                    # runs all k=0 matmuls first (they only need the first
                    # halves of the w1/xt DMAs).
                    h1a = h1apool.tile([128, 2, NT], F8, tag="h1a")
                    korder = ([(k, c) for k in range(2) for c in range(4)]
                              if t <= 1 else
                              [(k, c) for c in range(4) for k in range(2)])
                    for k, c in korder:
                        m0, mw = M1[c]
                        ps = ps1_t[c]
                        nc.tensor.matmul(
                            ps[0:mw, :],
                            w1_sb[:, k, :, ds(m0, mw)],
                            xt_t[:, k, :, :],
                            start=(k == 0),
                            stop=(k == 1),
                            perf_mode=DR,
                        )
                        if k != 1:
                            continue
                        # relu((8y1) + 8b1) -> 8*h1 in fp8; the small
                        # chunk goes to ACT to offload the DVE
                        if c < 2:
                            dest = h1a[:, c, :]
                        elif c == 2:
                            dest = h1b[:, 0, :]
                        else:
                            dest = h1b[0:32, 1, :]
                        if c < 3:
                            nc.vector.tensor_scalar(
                                dest, ps[0:mw, :], b1_sb[0:mw, c:c + 1], 0.0,
                                ALU.add, ALU.max
                            )
                        else:
                            nc.scalar.activation(
                                dest, ps[0:mw, :], AF.Relu,
                                bias=b1_sb[0:mw, c:c + 1])
                    fc1_out[t] = (h1a, h1b)
                    fc2_in[t] = (c1pool.tile([128, 2, NT], F8, tag="c1",
                                             name="c1"), c2)

                # head of tile t-2, interleaved between fc1(t) and fc2(t-1)
                if t >= 2:
                    emit_head_phase(t - 2)

                if 1 <= t <= n_tiles:
                    h1a_p, h1b_p = fc1_out.pop(t - 1)
                    c1_p, c2_p = fc2_in.pop(t - 1)
                    emit_fc2(t - 1, h1a_p, h1b_p, c1_p, c2_p)
                flush_epi(t)
            flush_out()

    nc.compile()
    return nc


def host_prep(frame, reward, last_action, eps, W1, b1, W2, b2, Wp, bp, Wb, bb,
              rows=R, n_cores=N_CORES):
    """Shard + lay out inputs for the device program. Returns in_maps."""
    frame = np.asarray(frame, np.float32).reshape(TB, OBS)
    reward = np.asarray(reward, np.float32).reshape(TB)
    la = np.asarray(last_action).reshape(TB).astype(np.float32)
    eps = np.asarray(eps, np.float32).reshape(TB)
    n_tiles = rows // NT

    W1 = np.asarray(W1, np.float32)
    W2 = np.asarray(W2, np.float32)
    b1 = np.asarray(b1, np.float32)
    b2 = np.asarray(b2, np.float32)
    Wp = np.asarray(Wp, np.float32)
    bp = np.asarray(bp, np.float32)
    Wb = np.asarray(Wb, np.float32)
    bb = np.asarray(bb, np.float32)

    # frame features f are split as f = 256k + 128j + ki
    frame_q = frame.astype(E4)          # one pass over the big tensor
    W1p = np.zeros((416, 512), np.float32)
    W1p[0:400] = 8.0 * W1
    w1_h = np.ascontiguousarray(
        W1p.T.reshape(2, 2, 128, 416).transpose(2, 0, 1, 3)).astype(E4)
    W2p = np.zeros((304, 512), np.float32)
    W2p[0:300, 0:400] = 8.0 * W2
    w2_h = np.ascontiguousarray(
        W2p.T.reshape(2, 2, 128, 304).transpose(2, 0, 1, 3)).astype(E4)

    # head weights, row-major heads: columns (mu, sigma, baseline, pad);
    # core rows: 0..255 (c1: f = 128j + ki), then c2 rows {0..43: h2
    # 256..299, 44..95: zero, 96: cr, 97: la, 98: const-8 bias row}
    Wh = np.concatenate([Wp, Wb], axis=0)           # [3, 302]
    bh = np.array([bp[0], bp[1], bb[0]], np.float32)
    wh1_h = np.zeros((128, 2, 4), np.float32)
    wh1_h[:, :, 0:3] = (8.0 * Wh[:, 0:256]).T.reshape(2, 128, 3).transpose(
        1, 0, 2)
    wh1_h = wh1_h.astype(E4)
    wh2_h = np.zeros((128, 4), np.float32)
    wh2_h[0:44, 0:3] = 8.0 * Wh[:, 256:300].T
    wh2_h[96:98, 0:3] = 8.0 * Wh[:, 300:302].T
    wh2_h[98, 0:3] = 8.0 * bh
    wh2_h = wh2_h.astype(E4)

    b1s = np.zeros(512, np.float32)
    b1s[0:400] = 8.0 * b1
    b1_h = np.ascontiguousarray(b1s.reshape(4, 128).T)
    b2s = np.zeros(384, np.float32)
    b2s[0:300] = 8.0 * b2
    b2_h = np.ascontiguousarray(b2s.reshape(3, 128).T)

    # fused constant blob (bytes), shared across cores except eps
    wbuf = np.zeros((128, WBYTES), np.uint8)
    wbuf[:, OFF_W1:OFF_W1 + 1664] = w1_h.reshape(128, 1664).view(np.uint8)
    wbuf[:, OFF_W2:OFF_W2 + 1216] = w2_h.reshape(128, 1216).view(np.uint8)
    wbuf[:, OFF_WH1:OFF_WH1 + 8] = wh1_h.reshape(128, 8).view(np.uint8)
    wbuf[:, OFF_WH2:OFF_WH2 + 4] = wh2_h.view(np.uint8)
    wbuf[:, OFF_B1:OFF_B1 + 16] = b1_h.view(np.uint8)
    wbuf[:, OFF_B2:OFF_B2 + 12] = b2_h.view(np.uint8)

    cr8 = (8.0 * np.clip(reward, -1.0, 1.0)).astype(E4)
    la8 = (8.0 * la).astype(E4)
    ones8 = np.full(TB, 8.0, np.float32).astype(E4)

    in_maps = []
    for c in range(n_cores):
        sl = slice(c * rows, (c + 1) * rows)
        xt = np.ascontiguousarray(
            frame_q[sl].T.reshape(2, 2, 128, n_tiles, NT)
            .transpose(2, 3, 0, 1, 4))
        rwla = np.stack([cr8[sl], la8[sl], ones8[sl]], axis=0)
        # eps row r lives at [r % 128, r // 128]
        eps_c = np.ascontiguousarray(eps[sl].reshape(rows // 128, 128).T)
        wb = wbuf.copy()
        wb[:, OFF_EPS:WBYTES] = eps_c.view(np.uint8)
        in_maps.append({
            "xt": xt,
            "rwla": rwla,
            "w8": wb.view(E4),
        })
    return in_maps


def assemble_out(per_core_outs):
    """[128, R//128, 4] per core (row r at [r%128, r//128]) -> [T, B, 4]."""
    outs = []
    for o in per_core_outs:
        o = np.asarray(o)
        outs.append(o.transpose(1, 0, 2).reshape(-1, B, 4))
    return np.ascontiguousarray(
        np.concatenate(outs, axis=0).astype(np.float32))


_NC_CACHE = {}


def kernel(**inputs) -> np.ndarray:
    in_maps = host_prep(**inputs)
    if R not in _NC_CACHE:
        _NC_CACHE[R] = build_bass(R)
    nc = _NC_CACHE[R]
    res = run_bass_kernel_spmd(nc, in_maps, core_ids=list(range(N_CORES)))
    return assemble_out([res.results[c]["out"] for c in range(N_CORES)])

